# revision 6
# baseline (speedup 1.0000x reference)
"""Multi-head attention (dense transformer block) on 8 Trainium2 NeuronCores.

Sharding: one attention head per core (H=8 heads, 8 cores), both batch
elements on every core; QKV/O weights are sliced per head on the host and
each core computes its head's full attention plus its partial contribution
to the output projection. Host sums the 8 partial projections (the only
cross-core reduction; there is no device-to-device communication).

v2 pipeline (all matmul inputs fp16, fp32 accumulation):
  - QKV: one fused matmul for [q;k] (stacked 128-wide lhsT) into a single
    PSUM tile; V is computed pre-transposed (x chunk as the stationary
    operand) so no separate PE transpose pass is needed.
  - Attention: software-pipelined per j-block: S(jb) matmul / exp(jb) on
    ACT / PV(jb-1) matmul are interleaved so PE and ACT overlap fully.
    The last PV + output projection of chunk c are spread into the first
    j-block slots of chunk c+1 to avoid ACT bubbles at chunk boundaries.
  - Softmax: P^T = exp(S^T/8 - SHIFT) (exact, global shift); the V_aug
    ones column accumulates l = sum P in PSUM row 64; normalization by
    1/l happens on DVE at projection evacuation.
"""
import numpy as np
from contextlib import ExitStack

import concourse.bass as bass
import concourse.tile as tile
from concourse import bacc, mybir
from concourse.bass_utils import run_bass_kernel_spmd

dt = mybir.dt

H = 8
HD = 64
D = 512
B = 2
N = 4096
SCALE = 0.125
SHIFT = 2.0  # global logit shift; exact for softmax, keeps exp() in fp16 range

MM_DT = dt.float16
MM_NP = np.float16


def _build(repeat=1, ichunk=1024, xt_bufs=4, pt_bufs=4):
    NB = B * N
    njb = N // 128
    nic = N // ichunk
    mm_per_ic = ichunk // 512

    nc = bacc.Bacc("TRN2", target_bir_lowering=False, debug=False, num_devices=8)
    xt = nc.dram_tensor("xt", [D, NB], MM_DT, kind="ExternalInput").ap()
    wqkvt = nc.dram_tensor("wqkvt", [D, 3 * HD], MM_DT, kind="ExternalInput").ap()
    woat = nc.dram_tensor("woat", [HD + 1, D], MM_DT, kind="ExternalInput").ap()
    part = nc.dram_tensor("part", [B, N, D], dt.float32, kind="ExternalOutput").ap()

    xtr = xt.rearrange("(d p) n -> p d n", p=128)  # [128, 4, NB]

    with tile.TileContext(nc) as tc:
        with ExitStack() as ctx:
            const_p = ctx.enter_context(tc.tile_pool(name="const", bufs=1))
            xt_p = ctx.enter_context(tc.tile_pool(name="xt", bufs=xt_bufs))
            qkv_p = ctx.enter_context(tc.tile_pool(name="qkv", bufs=1))
            vaug_p = ctx.enter_context(tc.tile_pool(name="vaug", bufs=1))
            pt_p = ctx.enter_context(tc.tile_pool(name="pt", bufs=pt_bufs))
            out_p = ctx.enter_context(tc.tile_pool(name="outs", bufs=6))
            ot_p = ctx.enter_context(tc.tile_pool(name="ot", bufs=4))
            small_p = ctx.enter_context(tc.tile_pool(name="small", bufs=8))
            spool = ctx.enter_context(tc.tile_pool(name="spool", bufs=2, space="PSUM"))
            opool = ctx.enter_context(tc.tile_pool(name="opool", bufs=3, space="PSUM"))
            projpool = ctx.enter_context(tc.tile_pool(name="projpool", bufs=1, space="PSUM"))

            shiftc = const_p.tile([128, 1], dt.float32, tag="shiftc")
            nc.vector.memset(shiftc[:], -SHIFT)
            # wq: [128, d-chunk, col] where cols 0:64 = q, 64:128 = k, 128:192 = v
            wq = const_p.tile([128, 4, 3 * HD], MM_DT, tag="wq")
            for d in range(4):
                nc.sync.dma_start(wq[:, d, :], wqkvt[d * 128:(d + 1) * 128, :])
            woa = const_p.tile([HD + 1, D], MM_DT, tag="woa")
            nc.sync.dma_start(woa[:], woat[:])

            # rows 0:64 = q^T (hd x tokens), rows 64:128 = k^T (staging)
            qkT = qkv_p.tile([128, NB], MM_DT, tag="qkT")
            # k^T relocated to partition base 0 (matmul needs lhsT/rhs same base)
            kT = qkv_p.tile([64, NB], MM_DT, tag="kT")
            vaug = [vaug_p.tile([128, njb, 65], MM_DT, tag=f"vaug{b}", name=f"vaug{b}")
                    for b in range(B)]

            state = {"pending": None}

            def qkv_phase(b):
                for ch in range(N // 512):
                    c0 = b * N + ch * 512
                    xts = xt_p.tile([128, 4, 512], MM_DT, tag="xt", name="xts")
                    nc.sync.dma_start(xts[:], xtr[:, :, c0:c0 + 512])
                    ps = spool.tile([128, 1024], dt.float32, tag="s", name="ps_qkv")
                    for d in range(4):
                        nc.tensor.matmul(ps[:, 0:512], wq[:, d, 0:128], xts[:, d, :],
                                         start=(d == 0), stop=(d == 3))
                    for t in range(4):
                        for d in range(4):
                            nc.tensor.matmul(
                                ps[:, 512 + t * 64:512 + (t + 1) * 64],
                                xts[:, d, t * 128:(t + 1) * 128],
                                wq[:, d, 128:192],
                                start=(d == 0), stop=(d == 3))
                    nc.vector.tensor_copy(qkT[:, c0:c0 + 512], ps[:, 0:512])
                    nc.vector.tensor_copy(vaug[b][:, ch * 4:(ch + 1) * 4, 0:64],
                                          ps[:, 512:768])
                    nc.sync.dma_start(kT[:, c0:c0 + 512], qkT[64:128, c0:c0 + 512])

            def make_pending(b, ic, ps_o, pt31):
                i0 = b * N + ic * ichunk

                def pending(stage):
                    if stage == 0:
                        for m in range(mm_per_ic):
                            nc.tensor.matmul(
                                ps_o[m][:], vaug[b][:, njb - 1, :],
                                pt31[:, m * 512:(m + 1) * 512],
                                start=False, stop=True)
                        return
                    m = stage - 1
                    ouT = ot_p.tile([65, 512], MM_DT, tag="ot", name="ouT")
                    nc.vector.tensor_copy(ouT[:], ps_o[m][:])
                    lrec = small_p.tile([65, 512], dt.float32, tag="lrec", name="lrec")
                    nc.vector.reciprocal(lrec[64:65, :], ps_o[m][64:65, :])
                    lrecT = small_p.tile([128, 4], dt.float32, tag="lrecT", name="lrecT")
                    for ib in range(4):
                        nc.sync.dma_start(
                            lrecT[:, ib:ib + 1],
                            lrec[64:65, ib * 128:(ib + 1) * 128])
                    for ib in range(4):
                        ps_p = projpool.tile([128, 512], dt.float32, tag="pj", name="ps_p")
                        nc.tensor.matmul(ps_p[:], ouT[:, ib * 128:(ib + 1) * 128],
                                         woa[:], start=True, stop=True)
                        osb = out_p.tile([128, 512], dt.float32, tag="ou", name="osb")
                        nc.vector.tensor_scalar_mul(osb[:], ps_p[:], lrecT[:, ib:ib + 1])
                        row0 = ic * ichunk + m * 512 + ib * 128
                        nc.sync.dma_start(part[b, row0:row0 + 128, :], osb[:])

                return pending

            def attn_chunk(b, ic):
                i0 = b * N + ic * ichunk
                drainp = state["pending"]
                ps_o = None
                prev_pt = None
                for jb in range(njb):
                    ps_s = spool.tile([128, ichunk], dt.float32, tag="s", name="ps_s")
                    for m in range(mm_per_ic):
                        nc.tensor.matmul(
                            ps_s[:, m * 512:(m + 1) * 512],
                            kT[:, b * N + jb * 128: b * N + (jb + 1) * 128],
                            qkT[0:64, i0 + m * 512: i0 + (m + 1) * 512],
                            start=True, stop=True)
                    ptile = pt_p.tile([128, ichunk], MM_DT, tag="pt", name="ptile")
                    nc.scalar.activation(
                        ptile[:], ps_s[:],
                        mybir.ActivationFunctionType.Exp,
                        bias=shiftc[:, 0:1], scale=SCALE)
                    if jb == 0:
                        if drainp is not None:
                            drainp(0)
                    else:
                        if jb == 1:
                            ps_o = [opool.tile([65, 512], dt.float32, tag="o", name="ps_o")
                                    for _ in range(mm_per_ic)]
                        for m in range(mm_per_ic):
                            nc.tensor.matmul(
                                ps_o[m][:], vaug[b][:, jb - 1, :],
                                prev_pt[:, m * 512:(m + 1) * 512],
                                start=(jb == 1), stop=False)
                        if drainp is not None and jb <= mm_per_ic:
                            drainp(jb)
                    prev_pt = ptile
                state["pending"] = make_pending(b, ic, ps_o, prev_pt)

            def body(_=None):
                for b in range(B):
                    nc.vector.memset(vaug[b][:, :, 64:65], 1.0)
                for b in range(B):
                    qkv_phase(b)
                    for ic in range(nic):
                        attn_chunk(b, ic)
                # drain the last chunk
                fin = state["pending"]
                state["pending"] = None
                for stage in range(mm_per_ic + 1):
                    fin(stage)

            if repeat == 1:
                body()
            else:
                with tc.For_i(0, repeat, 1) as _i:
                    body()

    nc.compile()
    return nc


def _make_in_maps(x, w_qkv, w_o, b_o):
    xt = np.ascontiguousarray(x.transpose(2, 1, 0).reshape(D, B * N)).astype(MM_NP)
    in_maps = []
    for c in range(8):
        wqs = w_qkv[c * HD:(c + 1) * HD]
        wks = w_qkv[D + c * HD:D + (c + 1) * HD]
        wvs = w_qkv[2 * D + c * HD:2 * D + (c + 1) * HD]
        wqkvt = np.ascontiguousarray(np.concatenate([wqs, wks, wvs], 0).T).astype(MM_NP)
        bo_row = b_o if c == 0 else np.zeros_like(b_o)
        woat = np.concatenate(
            [w_o[:, c * HD:(c + 1) * HD].T, bo_row[None, :]], 0).astype(MM_NP)
        in_maps.append({"xt": xt, "wqkvt": wqkvt, "woat": woat})
    return in_maps


_NC_CACHE = {}


def _get_nc(repeat=1, **kw):
    key = (repeat, tuple(sorted(kw.items())))
    if key not in _NC_CACHE:
        _NC_CACHE[key] = _build(repeat=repeat, **kw)
    return _NC_CACHE[key]


def kernel(x, w_qkv, w_o, b_o):
    x = np.asarray(x, np.float32)
    w_qkv = np.asarray(w_qkv, np.float32)
    w_o = np.asarray(w_o, np.float32)
    b_o = np.asarray(b_o, np.float32)
    assert x.shape == (N, B, D), x.shape
    nc = _get_nc()
    in_maps = _make_in_maps(x, w_qkv, w_o, b_o)
    res = run_bass_kernel_spmd(nc, in_maps, list(range(8)))
    acc = np.zeros((B, N, D), np.float64)
    for r in res.results:
        acc += r["part"]
    return acc.astype(np.float32)


# revision 12
# speedup vs baseline: 1.1182x; 1.1182x over previous
"""Multi-head attention (dense transformer block) on 8 Trainium2 NeuronCores.

Sharding: one attention head per core (H=8 heads, 8 cores), both batch
elements on every core; QKV/O weights are sliced per head on the host and
each core computes its head's full attention plus its partial contribution
to the output projection. Host sums the 8 partial projections (the only
cross-core reduction; there is no device-to-device communication).

v2 pipeline (all matmul inputs fp16, fp32 accumulation):
  - QKV: one fused matmul for [q;k] (stacked 128-wide lhsT) into a single
    PSUM tile; V is computed pre-transposed (x chunk as the stationary
    operand) so no separate PE transpose pass is needed.
  - Attention: software-pipelined per j-block: S(jb) matmul / exp(jb) on
    ACT / PV(jb-1) matmul are interleaved so PE and ACT overlap fully.
    The last PV + output projection of chunk c are spread into the first
    j-block slots of chunk c+1 to avoid ACT bubbles at chunk boundaries.
  - Softmax: P^T = exp(S^T/8 - SHIFT) (exact, global shift); the V_aug
    ones column accumulates l = sum P in PSUM row 64; normalization by
    1/l happens on DVE at projection evacuation.
"""
import numpy as np
from contextlib import ExitStack

import concourse.bass as bass
import concourse.tile as tile
from concourse import bacc, mybir
from concourse.bass_utils import run_bass_kernel_spmd

dt = mybir.dt

H = 8
HD = 64
D = 512
B = 2
N = 4096
SCALE = 0.125
SHIFT = 2.0  # global logit shift; exact for softmax, keeps exp() in fp16 range

MM_DT = dt.float16
MM_NP = np.float16


def _build(repeat=1, ichunk=1024, xt_bufs=4, pt_bufs=4, interleave_pv=True,
           v_direct=True):
    NB = B * N
    njb = N // 128
    nic = N // ichunk
    mm_per_ic = ichunk // 512
    if not interleave_pv:
        pt_bufs = max(pt_bufs, njb + 2)

    nc = bacc.Bacc("TRN2", target_bir_lowering=False, debug=False, num_devices=8)
    xt = nc.dram_tensor("xt", [D, NB], MM_DT, kind="ExternalInput").ap()
    wqkvt = nc.dram_tensor("wqkvt", [D, 3 * HD], MM_DT, kind="ExternalInput").ap()
    woat = nc.dram_tensor("woat", [HD + 1, D], MM_DT, kind="ExternalInput").ap()
    part = nc.dram_tensor("part", [B, N, D], dt.float32, kind="ExternalOutput").ap()

    xtr = xt.rearrange("(d p) n -> p d n", p=128)  # [128, 4, NB]

    with tile.TileContext(nc) as tc:
        with ExitStack() as ctx:
            const_p = ctx.enter_context(tc.tile_pool(name="const", bufs=1))
            xt_p = ctx.enter_context(tc.tile_pool(name="xt", bufs=xt_bufs))
            qkv_p = ctx.enter_context(tc.tile_pool(name="qkv", bufs=1))
            vaug_p = ctx.enter_context(tc.tile_pool(name="vaug", bufs=1))
            pt_p = ctx.enter_context(tc.tile_pool(name="pt", bufs=pt_bufs))
            out_p = ctx.enter_context(tc.tile_pool(name="outs", bufs=6))
            ot_p = ctx.enter_context(tc.tile_pool(name="ot", bufs=4))
            small_p = ctx.enter_context(tc.tile_pool(name="small", bufs=8))
            spool = ctx.enter_context(tc.tile_pool(name="spool", bufs=2, space="PSUM"))
            opool = ctx.enter_context(tc.tile_pool(name="opool", bufs=3, space="PSUM"))
            projpool = ctx.enter_context(tc.tile_pool(name="projpool", bufs=1, space="PSUM"))

            shiftc = const_p.tile([128, 1], dt.float32, tag="shiftc")
            nc.vector.memset(shiftc[:], -SHIFT)
            if not v_direct:
                from concourse.masks import make_identity
                ident = const_p.tile([128, 128], MM_DT, tag="ident")
                make_identity(nc, ident[:])
            # wq: [128, d-chunk, col] where cols 0:64 = q, 64:128 = k, 128:192 = v
            wq = const_p.tile([128, 4, 3 * HD], MM_DT, tag="wq")
            for d in range(4):
                nc.sync.dma_start(wq[:, d, :], wqkvt[d * 128:(d + 1) * 128, :])
            woa = const_p.tile([HD + 1, D], MM_DT, tag="woa")
            nc.sync.dma_start(woa[:], woat[:])

            # rows 0:64 = q^T (hd x tokens), rows 64:128 = k^T (staging)
            qkT = qkv_p.tile([128, NB], MM_DT, tag="qkT")
            # k^T relocated to partition base 0 (matmul needs lhsT/rhs same base)
            kT = qkv_p.tile([64, NB], MM_DT, tag="kT")
            vT = None if v_direct else qkv_p.tile([64, N], MM_DT, tag="vT")
            vaug = [vaug_p.tile([128, njb, 65], MM_DT, tag=f"vaug{b}", name=f"vaug{b}")
                    for b in range(B)]

            state = {"pending": None}

            def qkv_phase(b):
                for ch in range(N // 512):
                    c0 = b * N + ch * 512
                    xts = xt_p.tile([128, 4, 512], MM_DT, tag="xt", name="xts")
                    nc.sync.dma_start(xts[:], xtr[:, :, c0:c0 + 512])
                    ps = spool.tile([128, 1024], dt.float32, tag="s", name="ps_qkv")
                    for d in range(4):
                        nc.tensor.matmul(ps[:, 0:512], wq[:, d, 0:128], xts[:, d, :],
                                         start=(d == 0), stop=(d == 3))
                    if v_direct:
                        for t in range(4):
                            for d in range(4):
                                nc.tensor.matmul(
                                    ps[:, 512 + t * 64:512 + (t + 1) * 64],
                                    xts[:, d, t * 128:(t + 1) * 128],
                                    wq[:, d, 128:192],
                                    start=(d == 0), stop=(d == 3))
                        nc.vector.tensor_copy(vaug[b][:, ch * 4:(ch + 1) * 4, 0:64],
                                              ps[:, 512:768])
                    else:
                        for d in range(4):
                            nc.tensor.matmul(ps[0:64, 512:1024], wq[:, d, 128:192],
                                             xts[:, d, :],
                                             start=(d == 0), stop=(d == 3))
                        nc.vector.tensor_copy(vT[:, c0 - b * N:c0 - b * N + 512],
                                              ps[0:64, 512:1024])
                    nc.vector.tensor_copy(qkT[:, c0:c0 + 512], ps[:, 0:512])
                    nc.sync.dma_start(kT[:, c0:c0 + 512], qkT[64:128, c0:c0 + 512])
                if not v_direct:
                    for jb in range(njb):
                        pt = opool.tile([128, 64], MM_DT, tag="o", name="pt_tr")
                        nc.tensor.transpose(
                            pt[:], vT[:, jb * 128:(jb + 1) * 128],
                            ident[0:64, 0:64])
                        nc.vector.tensor_copy(vaug[b][:, jb, 0:64], pt[:])

            def make_pending(b, ic, ps_o, pt31, pv_last):
                def pending(stage):
                    if stage == 0:
                        if pv_last:
                            for m in range(mm_per_ic):
                                nc.tensor.matmul(
                                    ps_o[m][:], vaug[b][:, njb - 1, :],
                                    pt31[:, m * 512:(m + 1) * 512],
                                    start=False, stop=True)
                        return
                    m = stage - 1
                    ouT = ot_p.tile([65, 512], MM_DT, tag="ot", name="ouT")
                    nc.vector.tensor_copy(ouT[:], ps_o[m][:])
                    lrec = small_p.tile([65, 512], dt.float32, tag="lrec", name="lrec")
                    nc.vector.reciprocal(lrec[64:65, :], ps_o[m][64:65, :])
                    lrecT = small_p.tile([128, 4], dt.float32, tag="lrecT", name="lrecT")
                    for ib in range(4):
                        nc.sync.dma_start(
                            lrecT[:, ib:ib + 1],
                            lrec[64:65, ib * 128:(ib + 1) * 128])
                    for ib in range(4):
                        ps_p = projpool.tile([128, 512], dt.float32, tag="pj", name="ps_p")
                        nc.tensor.matmul(ps_p[:], ouT[:, ib * 128:(ib + 1) * 128],
                                         woa[:], start=True, stop=True)
                        osb = out_p.tile([128, 512], dt.float32, tag="ou", name="osb")
                        nc.vector.tensor_scalar_mul(osb[:], ps_p[:], lrecT[:, ib:ib + 1])
                        row0 = ic * ichunk + m * 512 + ib * 128
                        nc.sync.dma_start(part[b, row0:row0 + 128, :], osb[:])

                return pending

            def attn_chunk(b, ic):
                i0 = b * N + ic * ichunk
                drainp = state["pending"]
                ps_o = None
                prev_pt = None
                pts = []
                for jb in range(njb):
                    ps_s = spool.tile([128, ichunk], dt.float32, tag="s", name="ps_s")
                    for m in range(mm_per_ic):
                        nc.tensor.matmul(
                            ps_s[:, m * 512:(m + 1) * 512],
                            kT[:, b * N + jb * 128: b * N + (jb + 1) * 128],
                            qkT[0:64, i0 + m * 512: i0 + (m + 1) * 512],
                            start=True, stop=True)
                    ptile = pt_p.tile([128, ichunk], MM_DT, tag="pt", name="ptile")
                    nc.scalar.activation(
                        ptile[:], ps_s[:],
                        mybir.ActivationFunctionType.Exp,
                        bias=shiftc[:, 0:1], scale=SCALE)
                    pts.append(ptile)
                    if jb == 0:
                        if drainp is not None:
                            drainp(0)
                    elif interleave_pv:
                        if jb == 1:
                            ps_o = [opool.tile([65, 512], dt.float32, tag="o", name="ps_o")
                                    for _ in range(mm_per_ic)]
                        for m in range(mm_per_ic):
                            nc.tensor.matmul(
                                ps_o[m][:], vaug[b][:, jb - 1, :],
                                prev_pt[:, m * 512:(m + 1) * 512],
                                start=(jb == 1), stop=False)
                        if drainp is not None and jb <= mm_per_ic:
                            drainp(jb)
                    elif drainp is not None and jb <= mm_per_ic:
                        drainp(jb)
                    prev_pt = ptile
                if interleave_pv:
                    state["pending"] = make_pending(b, ic, ps_o, prev_pt, True)
                else:
                    ps_o = [opool.tile([65, 512], dt.float32, tag="o", name="ps_o")
                            for _ in range(mm_per_ic)]
                    for jb in range(njb):
                        for m in range(mm_per_ic):
                            nc.tensor.matmul(
                                ps_o[m][:], vaug[b][:, jb, :],
                                pts[jb][:, m * 512:(m + 1) * 512],
                                start=(jb == 0), stop=(jb == njb - 1))
                    state["pending"] = make_pending(b, ic, ps_o, None, False)

            def body(_=None):
                for b in range(B):
                    nc.vector.memset(vaug[b][:, :, 64:65], 1.0)
                for b in range(B):
                    qkv_phase(b)
                    for ic in range(nic):
                        attn_chunk(b, ic)
                # drain the last chunk
                fin = state["pending"]
                state["pending"] = None
                for stage in range(mm_per_ic + 1):
                    fin(stage)

            if repeat == 1:
                body()
            else:
                with tc.For_i(0, repeat, 1) as _i:
                    body()

    nc.compile()
    return nc


def _make_in_maps(x, w_qkv, w_o, b_o):
    xt = np.ascontiguousarray(x.transpose(2, 1, 0).reshape(D, B * N)).astype(MM_NP)
    in_maps = []
    for c in range(8):
        wqs = w_qkv[c * HD:(c + 1) * HD]
        wks = w_qkv[D + c * HD:D + (c + 1) * HD]
        wvs = w_qkv[2 * D + c * HD:2 * D + (c + 1) * HD]
        wqkvt = np.ascontiguousarray(np.concatenate([wqs, wks, wvs], 0).T).astype(MM_NP)
        bo_row = b_o if c == 0 else np.zeros_like(b_o)
        woat = np.concatenate(
            [w_o[:, c * HD:(c + 1) * HD].T, bo_row[None, :]], 0).astype(MM_NP)
        in_maps.append({"xt": xt, "wqkvt": wqkvt, "woat": woat})
    return in_maps


_NC_CACHE = {}


def _get_nc(repeat=1, **kw):
    key = (repeat, tuple(sorted(kw.items())))
    if key not in _NC_CACHE:
        _NC_CACHE[key] = _build(repeat=repeat, **kw)
    return _NC_CACHE[key]


def kernel(x, w_qkv, w_o, b_o):
    x = np.asarray(x, np.float32)
    w_qkv = np.asarray(w_qkv, np.float32)
    w_o = np.asarray(w_o, np.float32)
    b_o = np.asarray(b_o, np.float32)
    assert x.shape == (N, B, D), x.shape
    nc = _get_nc()
    in_maps = _make_in_maps(x, w_qkv, w_o, b_o)
    res = run_bass_kernel_spmd(nc, in_maps, list(range(8)))
    acc = np.zeros((B, N, D), np.float64)
    for r in res.results:
        acc += r["part"]
    return acc.astype(np.float32)
